# revision 24
# baseline (speedup 1.0000x reference)
"""Multi-head attention block on 8 Trainium2 NeuronCores, data-parallel over batch.

Per core (one batch element, S=1024 seq, E=1024 embed, H=16 heads, D=64):
  xT = transpose(x)                      (PE transpose, 64 x [128,128] tiles)
  qT/kT = W_pair.T @ xT  (feature-major) per head-pair, float32r matmuls
  V     = xT.T @ Wv      (seq-major) with a ones column appended -> V_aug [S, h, 65]
  scoresT[s2,s1] = kT.T @ qT  (two heads packed as K=64 row-tiles, concurrent on PE)
  expT = exp(0.125*scoresT)   (ACT eviction PSUM->SBUF, softmax without max-subtract)
  PV: psum[65,512] = V_aug.T @ expT  -> rows 0..63 = outT unnormalized, row 64 = rowsum
  normalize: outT = psum[0:64] * broadcast(1/psum[64])   (DVE + GPSIMD broadcast)
  out = outT.T @ W_out + b_out            (float32r)

Weights are de-interleaved host-side: reference W_qkv columns are (h, d, qkv)
with qkv innermost; we feed wqk (pair-blocked [q0q1k0k1...]) and wv ((h,d) order).
"""

import ml_dtypes
import numpy as np

import concourse.bacc as bacc
import concourse.bass as bass
import concourse.mybir as mybir
from concourse.bass_utils import run_bass_kernel_spmd
from concourse.masks import make_identity
from concourse.tile import TileContext
from concourse.tile_rust import add_dep_helper

F32 = mybir.dt.float32
R32 = mybir.dt.float32r
BF16 = mybir.dt.bfloat16
AF = mybir.ActivationFunctionType

S = 1024       # sequence length
E = 1024       # embed dim
H = 16         # heads
D = 64         # head dim
P = 128        # partitions
NP = 8         # head pairs
KT = E // P    # contraction tiles (8)
SM = S // P    # seq tiles of 128 (8)
NB = S // 512  # seq banks of 512 (2)
SCALE = 1.0 / np.sqrt(D)


def r(ap):
    return ap


def build_nc():
    nc = bacc.Bacc(trn_type="TRN2", target_bir_lowering=False)
    x = nc.dram_tensor("x", [S, E], BF16, kind="ExternalInput")
    wqk = nc.dram_tensor("wqk", [E, 2 * E], BF16, kind="ExternalInput")
    wv = nc.dram_tensor("wv", [E, E], BF16, kind="ExternalInput")
    bqk = nc.dram_tensor("bqk", [2 * E], F32, kind="ExternalInput")
    bv = nc.dram_tensor("bv", [E], F32, kind="ExternalInput")
    wout = nc.dram_tensor("wout", [E, E], BF16, kind="ExternalInput")
    bout = nc.dram_tensor("bout", [E], F32, kind="ExternalInput")
    out = nc.dram_tensor("out", [S, E], F32, kind="ExternalOutput")

    with TileContext(nc) as tc:
        with (
            tc.tile_pool(name="const", bufs=1) as constp,
            tc.tile_pool(name="persist", bufs=1) as pers,
            tc.tile_pool(name="psum", bufs=1, space="PSUM") as psp,
        ):
            # ---- constants ----
            identity = constp.tile([P, P], BF16, tag="ident")
            make_identity(nc, identity)
            ones = constp.tile([1, 512], F32, tag="ones")
            nc.vector.memset(ones[:], 1.0)
            onespp = constp.tile([P, 2 * H], F32, tag="onespp")
            nc.vector.memset(onespp[:], 1.0)
            # per-partition bias columns for q/k (column p*2+j is the bias for
            # pair p's q (j=0) / k (j=1) feature block)
            bcols = constp.tile([P, 2 * NP], F32, tag="bcols")
            nc.sync.dma_start(bcols[:], bqk.ap().rearrange("(f p) -> p f", p=P))

            # ---- persistent arrays ----
            xT = [pers.tile([P, S], BF16, tag=f"xt{k}", name=f"xT{k}") for k in range(KT)]
            vaug = [pers.tile([P, H, D + 2], BF16, tag=f"va{m}", name=f"vaug{m}")
                    for m in range(SM)]
            outT = [pers.tile([P, S], BF16, tag=f"ot{p}", name=f"outT{p}")
                    for p in range(NP)]

            # broadcast biases for free-dim adds (V and final projections)
            bvb = constp.tile([P, E], F32, tag="bvb")
            boutb = constp.tile([P, E], F32, tag="boutb")
            with (
                tc.tile_pool(name="ph0", bufs=1) as ph0,
                tc.tile_pool(name="ph2", bufs=1) as ph2,
                tc.tile_pool(name="ph3", bufs=1) as ph3,
            ):
                def load_wq(p):
                    wq = []
                    for k in range(KT):
                        w = ph2.tile([P, 256], BF16, tag="wqk", bufs=16, name="wqk")
                        nc.sync.dma_start(
                            w[:], wqk.ap()[bass.ts(k, P), bass.ts(p, 256)])
                        wq.append(w)
                    return wq
                _wq0 = {}
                bvr = ph0.tile([1, E], F32, tag="bvr")
                nc.sync.dma_start(bvr[:], bv.ap()[None, :])
                botr = ph0.tile([1, E], F32, tag="botr")
                nc.sync.dma_start(botr[:], bout.ap()[None, :])
                for n in range(2):
                    cs = bass.ts(n, 512)
                    pb = psp.tile([P, 512], F32, tag="mm", bufs=2, name="pb")
                    nc.tensor.matmul(pb[:], ones[0:1, 0:P], bvr[0:1, cs])
                    nc.vector.tensor_copy(bvb[:, cs], pb[:])
                    pb2 = psp.tile([P, 512], F32, tag="mm", bufs=2, name="pb2")
                    nc.tensor.matmul(pb2[:], ones[0:1, 0:P], botr[0:1, cs])
                    nc.vector.tensor_copy(boutb[:, cs], pb2[:])

                # ---- phase 0: transpose x into xT ----
                for m in range(SM):
                    xs = ph0.tile([P, E], BF16, tag="xs", bufs=3, name="xs")
                    nc.sync.dma_start(xs[:], x.ap()[bass.ts(m, P), :])
                    for k in range(KT):
                        tp = psp.tile([P, P], BF16, tag="pv", bufs=2, name="tp")
                        nc.tensor.transpose(
                            tp[:], xs[:, bass.ts(k, P)], identity[:])
                        nc.vector.tensor_copy(xT[k][:, bass.ts(m, P)], tp[:])

                # ---- phase 1: V = x @ Wv (+bv), into vaug with ones column ----
                for m in range(SM):
                    nc.vector.tensor_copy(
                        vaug[m][:, :, D:D + 2],
                        onespp[:].rearrange("p (h t) -> p h t", h=H))
                for n in range(2):
                    wvk = []
                    for k in range(KT):
                        w = ph0.tile([P, 512], BF16, tag=f"wv{k}", bufs=2,
                                     name="wvk")
                        nc.sync.dma_start(
                            w[:], wv.ap()[bass.ts(k, P), bass.ts(n, 512)])
                        wvk.append(w)
                    if n == 1:
                        _wq0["wq"] = load_wq(0)
                    for m in range(SM):
                        pv = psp.tile([P, 512], F32, tag="mm", bufs=2, name="pvps")
                        for k in range(KT):
                            nc.tensor.matmul(
                                pv[:], r(xT[k][:, bass.ts(m, P)]), r(wvk[k][:]),
                                start=(k == 0), stop=(k == KT - 1))
                        nc.vector.tensor_add(
                            vaug[m][:, bass.ts(n, 8), 0:D],
                            pv[:].rearrange("p (h d) -> p h d", h=8),
                            bvb[:, bass.ts(n, 512)].rearrange("p (h d) -> p h d", h=8))

                # ---- phase 2: attention, software-pipelined over head pairs ----
                # Iteration p computes attention for pair p while projecting
                # qt/kt for pair p+1 (proj matmuls interleaved into the scores
                # loop so PE has independent work while ACT evicts exp tiles).
                def load_wot(n):
                    cs = bass.ts(n, 512)
                    wot = []
                    for k in range(KT):
                        w = ph3.tile([P, 512], BF16, tag=f"wo{k}", bufs=2,
                                     name="wot")
                        nc.sync.dma_start(w[:], wout.ap()[bass.ts(k, P), cs])
                        wot.append(w)
                    return wot

                def alloc_qkt():
                    qt = ph2.tile([P, S], BF16, tag="qt", bufs=2, name="qt")
                    kt = ph2.tile([P, S], BF16, tag="kt", bufs=2, name="kt")
                    return qt, kt

                def proj_mms(p, wq, qt, kt):
                    """Generator yielding groups of proj matmuls + evictions."""
                    for which in range(2):  # 0 = q, 1 = k
                        ws = slice(which * P, (which + 1) * P)
                        dst = qt if which == 0 else kt
                        bc = bcols[:, 2 * p + which:2 * p + which + 1]
                        for n in range(NB):
                            cs = bass.ts(n, 512)
                            ps = psp.tile([P, 512], F32, tag="mm", bufs=2,
                                          name="pproj")
                            for k in range(KT):
                                nc.tensor.matmul(
                                    ps[:], wq[k][:, ws], xT[k][:, cs],
                                    start=(k == 0), stop=(k == KT - 1))
                                yield
                            nc.vector.tensor_scalar_add(dst[:, cs], ps[:], bc)
                    while True:
                        yield

                def drain(gen, n):
                    for _ in range(n):
                        next(gen)

                wq = _wq0["wq"]
                qt, kt = alloc_qkt()
                drain(proj_mms(0, wq, qt, kt), 40)

                for p in range(NP):
                    if p + 1 < NP:
                        wq_n = load_wq(p + 1)
                        qt_n, kt_n = alloc_qkt()
                        filler = proj_mms(p + 1, wq_n, qt_n, kt_n)
                    else:
                        def _noop():
                            while True:
                                yield
                        filler = _noop()

                    for n in range(NB):
                        cs = bass.ts(n, 512)
                        expA = ph2.tile([P, SM, 512], BF16, tag="expA", bufs=2, name="expA")
                        expB = ph2.tile([P, SM, 512], BF16, tag="expB", bufs=2, name="expB")
                        poA = psp.tile([D + 2, 512], F32, tag="pv", bufs=2,
                                       name="poA")
                        poB = psp.tile([D + 2, 512], F32, tag="pv", bufs=2,
                                       name="poB")
                        def emit_pv(m):
                            for j in range(2):
                                nc.tensor.matmul(
                                    poA[:], vaug[m + j][:, 2 * p, :],
                                    expA[:, m + j],
                                    start=(m + j == 0), stop=(m + j == SM - 1))
                                nc.tensor.matmul(
                                    poB[:], vaug[m + j][:, 2 * p + 1, :],
                                    expB[:, m + j],
                                    start=(m + j == 0), stop=(m + j == SM - 1))

                        for m in range(0, SM, 2):
                            psA = psp.tile([P, 2, 512], F32, tag="sc", bufs=2,
                                           name="psA")
                            psB = psp.tile([P, 2, 512], F32, tag="sc", bufs=2,
                                           name="psB")
                            prev = None
                            for j in range(2):
                                ms = bass.ts(m + j, P)
                                ia = nc.tensor.matmul(
                                    psA[:, j], kt[0:D, ms], qt[0:D, cs])
                                ib = nc.tensor.matmul(
                                    psB[:, j], kt[D:P, ms], qt[D:P, cs])
                                # chain so the two half-array (row-tiled)
                                # matmuls issue back-to-back and overlap
                                if prev is not None:
                                    add_dep_helper(ia.ins, prev.ins, sync=False,
                                                   reason="pair scores order")
                                add_dep_helper(ib.ins, ia.ins, sync=False,
                                               reason="pair scores order")
                                prev = ib
                            nc.scalar.activation(
                                expA[:, m:m + 2], psA[:], AF.Exp, scale=SCALE)
                            nc.scalar.activation(
                                expB[:, m:m + 2], psB[:], AF.Exp, scale=SCALE)
                            drain(filler, 2)
                        for m in range(0, SM, 2):
                            emit_pv(m)
                        for h, po in ((0, poA), (1, poB)):
                            # evict [66,512] to SBUF fast so the PSUM slot
                            # frees; normalize out of SBUF
                            pvt = ph2.tile([D, 512], F32, tag="pvt",
                                           bufs=4, name="pvt")
                            nc.scalar.copy(pvt[:], po[0:D, :])
                            rs = ph2.tile([1, 512], F32, tag="rs", bufs=4,
                                          name="rs")
                            nc.vector.tensor_copy(rs[:], po[D:D + 1, :])
                            drain(filler, 4)
                            rec = ph2.tile([1, 512], F32, tag="rec", bufs=4,
                                           name="rec")
                            nc.vector.reciprocal_approx_fast(rec[:], rs[:])
                            rb = ph2.tile([D, 512], F32, tag="rb", bufs=4,
                                          name="rb")
                            nc.gpsimd.partition_broadcast(rb[:], rec[:])
                            nc.vector.tensor_mul(
                                outT[p][h * D:(h + 1) * D, cs],
                                pvt[:], rb[:])
                    if p + 1 < NP:
                        drain(filler, 64)
                        wq, qt, kt = wq_n, qt_n, kt_n
                    if p == NP - 2:
                        wot0 = load_wot(0)

                # ---- phase 3: out = outT.T @ W_out + b_out ----
                for n in range(2):
                    cs = bass.ts(n, 512)
                    wot = wot0 if n == 0 else load_wot(1)
                    for m in range(SM):
                        pf = psp.tile([P, 512], F32, tag="mm", bufs=2, name="pf")
                        for k in range(KT):
                            nc.tensor.matmul(
                                pf[:], r(outT[k][:, bass.ts(m, P)]), r(wot[k][:]),
                                start=(k == 0), stop=(k == KT - 1))
                        osb = ph3.tile([P, 512], F32, tag="osb", bufs=3,
                                       name="osb")
                        nc.vector.tensor_add(osb[:], pf[:], boutb[:, cs])
                        nc.sync.dma_start(out.ap()[bass.ts(m, P), cs], osb[:])

    nc.finalize()
    return nc


_NC = None


def _get_nc():
    global _NC
    if _NC is None:
        _NC = build_nc()
    return _NC


def _prep_weights(W_qkv, b_qkv):
    # reference column order is (h, d, qkv) with qkv innermost
    W = np.asarray(W_qkv, dtype=np.float32).reshape(E, H, D, 3)
    b = np.asarray(b_qkv, dtype=np.float32).reshape(H, D, 3)
    Wq = W[..., 0].reshape(E, E)
    Wk = W[..., 1].reshape(E, E)
    Wv = W[..., 2].reshape(E, E)
    bq = b[..., 0].reshape(E)
    bk = b[..., 1].reshape(E)
    bv = b[..., 2].reshape(E)
    wqk = np.empty((E, 2 * E), dtype=np.float32)
    bqk = np.empty(2 * E, dtype=np.float32)
    for p in range(NP):
        wqk[:, p * 256:p * 256 + P] = Wq[:, p * P:(p + 1) * P]
        wqk[:, p * 256 + P:(p + 1) * 256] = Wk[:, p * P:(p + 1) * P]
        bqk[p * 256:p * 256 + P] = bq[p * P:(p + 1) * P]
        bqk[p * 256 + P:(p + 1) * 256] = bk[p * P:(p + 1) * P]
    return wqk, np.ascontiguousarray(Wv), bqk, bv


def kernel(x, W_qkv, b_qkv, W_out, b_out, _trace=False, _tmpdir=None):
    bf = ml_dtypes.bfloat16
    x = np.ascontiguousarray(np.asarray(x, dtype=np.float32).astype(bf))
    wqk, wv, bqk, bv = _prep_weights(W_qkv, b_qkv)
    wqk = wqk.astype(bf)
    wv = wv.astype(bf)
    wout = np.ascontiguousarray(
        np.asarray(W_out, dtype=np.float32).astype(bf))
    bout = np.ascontiguousarray(np.asarray(b_out, dtype=np.float32))
    nc = _get_nc()
    in_maps = [
        {"x": np.ascontiguousarray(x[i]), "wqk": wqk, "wv": wv, "bqk": bqk,
         "bv": bv, "wout": wout, "bout": bout}
        for i in range(x.shape[0])
    ]
    res = run_bass_kernel_spmd(
        nc, in_maps, core_ids=list(range(x.shape[0])),
        trace=_trace, tmpdir=_tmpdir)
    outp = np.stack([rr["out"] for rr in res.results], axis=0)
    kernel.last_result = res
    return outp
